# revision 7
# baseline (speedup 1.0000x reference)
"""Trainium2 Bass kernel for nn_Conv2Central (S^4 separable stencil), v2.

The reference computes y = S(rev(S(S(rev(S(x)))))) where S is the 2x2
stencil  out[i,j] = x[i,j] + .5 x[i,j+1] + .5 x[i+1,j] + .25 x[i+1,j+1]
(zero-padded bottom/right) applied per image, and rev reverses the batch.
S commutes with batch permutations, so the flips cancel: the net op is
S^4 — a separable 5-tap filter K5 = [1, 2, 1.5, 0.5, 0.0625] applied
along H then W (zero-padded bottom/right).

Sharding: batch N=32 split across 8 NeuronCores (4 images/core), no
inter-core communication.

v2 strategy (DMA-roofline bound, fp16 I/O):
  - I/O in fp16 (host casts; rounding ~2^-11 ≪ the 2e-2 gate), which
    halves HBM traffic vs fp32: ~16 MiB/core -> ~52 us at ~330 GB/s.
  - Horizontal K5 = [1,1,0.25] ⊛ [1,1,0.25] ([1,.5]^2 squared):
      t = x + x(+1)          DVE tensor_add (fp16 2x_1p mode)
      u = 0.25 * x(+2)       exact pow2 scale, split ACT (scalar.mul)
                             and Pool (gpsimd.tensor_scalar_mul)
      g = t + u              DVE tensor_add
  - Vertical K5 x horizontal [1,1,0.25] on PE: 3 banded fp16 matmuls
    per 512-col half accumulating in fp32 PSUM, stationary weights
    V_c and 0.25*V_c (V[k,m] = K5[k-m], zeroed across image
    boundaries).
  - PSUM -> SBUF fp16 copy split across ACT and Pool, then DMA out.
Tiles: 128 input rows -> 124 output rows (stride 124), final tile 128.
"""
import numpy as np

import concourse.bass as bass
import concourse.mybir as mybir
from concourse.tile import TileContext
from concourse.bass_utils import run_bass_kernel_spmd
from bass_rust import ScopedClock

N_CORES = 8
B = 4            # images per core
H = 1024
W = 1024
STRIDE = 124
K5 = [1.0, 2.0, 1.5, 0.5, 0.0625]
U_ACT_COLS = 400     # of W+2 u-columns, how many ACT computes (rest Pool)
CP_ACT_COLS = 924    # of W output columns ACT copies (rest DVE; Pool
                     # cannot read PSUM)

# ---------------------------------------------------------------------------
# Workarounds for this container's walrus build, which rejects any
# instruction carrying more than ONE sync wait ("Too many sync wait
# commands").  (1) TileContext's tail drain aggregates a wait per live
# semaphore — replace it with a chain of sync NOPs, one wait each.
# (2) A general pass splits any remaining multi-wait instruction by
# hoisting extra waits onto same-engine NoOps inserted right before it
# (engine queues are FIFO, so the waits still complete first).
# ---------------------------------------------------------------------------


def _patched_drain_and_barrier(self, tick_clock, wait_clock):
    nc = self.nc
    probe = nc.sync.nop()
    wait_clock.add_sem_waits(probe.ins, ScopedClock({None: tick_clock.global_clock}))
    si = probe.ins.sync_info
    waits = list(si.on_wait) if si and si.on_wait else []
    if si is not None:
        si.on_wait = waits[:1]
    for i in range(1, len(waits)):
        n = nc.sync.nop()
        nsi = n.ins.sync_info
        if nsi is None:
            n.ins.sync_info = mybir.SyncInfo(on_wait=[waits[i]], on_update=[])
        else:
            nsi.on_wait = [waits[i]]
    nc.sync.drain()
    nc.all_engine_barrier()
    assert self.sems is not None
    popped = nc._tile_sem_poison_stack.pop()
    assert popped is self._sem_poison
    nc.clear_and_free_semaphores(list(self.sems.allocated().values()))
    nc.all_engine_barrier()


TileContext._drain_and_barrier = _patched_drain_and_barrier

_nop_counter = [0]


def _legalize_waits(nc):
    for f in nc.m.functions:
        for blk in f.blocks:
            out = []
            for inst in blk.instructions:
                si = inst.sync_info
                waits = list(si.on_wait) if si is not None and si.on_wait else []
                if len(waits) > 1:
                    for w in waits[:-1]:
                        _nop_counter[0] += 1
                        nop = mybir.InstNoOp(name=f"legalize-wait-{_nop_counter[0]}")
                        nop.engine = inst.engine
                        nop.sync_info = mybir.SyncInfo(on_wait=[w], on_update=[])
                        out.append(nop)
                    si.on_wait = [waits[-1]]
                out.append(inst)
            blk.instructions = out
    return nc


# ---------------------------------------------------------------------------
# Weights: banded vertical-filter matrices.
# ---------------------------------------------------------------------------


def _band_np(rows_in, rows_out, boundary=None):
    """A[k, m] = K5[k-m], zeroed where out-row m and in-row k straddle
    `boundary` (tile-local image split)."""
    A = np.zeros((rows_in, rows_out), dtype=np.float32)
    for m in range(rows_out):
        for d in range(5):
            k = m + d
            if k < rows_in and not (boundary is not None and m < boundary <= k):
                A[k, m] = K5[d]
    return A


def _tile_plan():
    """[(r0, pin, pout, boundary_or_None)] covering B*H rows."""
    total = B * H
    plan = []
    r0 = 0
    while r0 < total:
        if total - r0 <= 128:
            plan.append((r0, total - r0, total - r0, None))
            break
        boundary = None
        for k in range(1, B):
            if r0 < k * H < r0 + 128:
                boundary = k * H - r0
        plan.append((r0, 128, STRIDE, boundary))
        r0 += STRIDE
    return plan


def _weights_np():
    plan = _tile_plan()
    classes = sorted({b for (_, _, _, b) in plan if b is not None})
    cols = []
    offs = {}

    def add(name, arr):
        offs[name] = sum(c.shape[1] for c in cols)
        cols.append(arr)

    add("main", _band_np(128, 128))
    add("mainq", 0.25 * _band_np(128, 128))
    for b in classes:
        add(f"main{b}", _band_np(128, STRIDE, boundary=b))
        add(f"mainq{b}", 0.25 * _band_np(128, STRIDE, boundary=b))
    return np.concatenate(cols, axis=1).astype(np.float16), offs


# ---------------------------------------------------------------------------
# Kernel builder.
# ---------------------------------------------------------------------------


def _build(reps=1, legalize=True):
    nc = bass.Bass(trn_type="TRN2")
    DT = mybir.dt.float16
    F32 = mybir.dt.float32
    pack, offs = _weights_np()
    x = nc.dram_tensor("x", [B, H, W], DT, kind="ExternalInput")
    wp = nc.dram_tensor("wpack", list(pack.shape), DT, kind="ExternalInput")
    y = nc.dram_tensor("y", [B, H, W], DT, kind="ExternalOutput")
    xf = x.rearrange("b h w -> (b h) w")
    yf = y.rearrange("b h w -> (b h) w")
    if reps > 1:
        scratch = nc.dram_tensor("scratch", [B, H, W], DT, kind="ExternalOutput")
        sf = scratch.rearrange("b h w -> (b h) w")

    plan = _tile_plan()
    UA, CA = U_ACT_COLS, CP_ACT_COLS

    with TileContext(nc) as tc:
        with tc.tile_pool(name="wpool", bufs=1) as wpool, \
             tc.tile_pool(name="xp", bufs=8) as xp, \
             tc.tile_pool(name="hp", bufs=9) as hp, \
             tc.tile_pool(name="op", bufs=8) as op, \
             tc.tile_pool(name="pp", bufs=4, space="PSUM") as pp:
            wt = wpool.tile(list(pack.shape), DT)
            nc.sync.dma_start(out=wt[:], in_=wp[:])

            def wslice(name, pin, pout):
                o = offs[name]
                return wt[:pin, o:o + pout]

            for rep in range(reps):
              of = yf if rep == 0 else sf
              for ti, (r0, pin, pout, bnd) in enumerate(plan):
                xt = xp.tile([128, W + 4], DT, tag="xt")
                nc.sync.dma_start(out=xt[:pin, 0:W], in_=xf[r0:r0 + pin, :])
                nc.vector.memset(xt[:pin, W:W + 4], 0)
                t = hp.tile([128, W + 2], DT, tag="t")
                u = hp.tile([128, W + 2], DT, tag="u")
                g = hp.tile([128, W + 2], DT, tag="g")
                # u = 0.25 * x(+2), exact pow2 scale; split ACT / Pool
                nc.scalar.mul(u[:pin, 0:UA], xt[:pin, 2:2 + UA], 0.25)
                nc.gpsimd.tensor_scalar_mul(
                    u[:pin, UA:W + 2], xt[:pin, 2 + UA:W + 4], 0.25)
                # t = x + x(+1); g = t + u  (fp16 2x_1p adds on DVE)
                nc.vector.tensor_add(
                    t[:pin, 0:W + 2], xt[:pin, 0:W + 2], xt[:pin, 1:W + 3])
                nc.vector.tensor_add(g[:pin], t[:pin], u[:pin])
                mname = "main" if bnd is None else f"main{bnd}"
                qname = "mainq" if bnd is None else f"mainq{bnd}"
                ps = pp.tile([128, W], F32, tag="ps")
                for h in range(2):
                    c0 = h * 512
                    nc.tensor.matmul(ps[:pout, c0:c0 + 512],
                                     wslice(mname, pin, pout),
                                     g[:pin, c0:c0 + 512],
                                     start=True, stop=False)
                    nc.tensor.matmul(ps[:pout, c0:c0 + 512],
                                     wslice(mname, pin, pout),
                                     g[:pin, c0 + 1:c0 + 513],
                                     start=False, stop=False)
                    nc.tensor.matmul(ps[:pout, c0:c0 + 512],
                                     wslice(qname, pin, pout),
                                     g[:pin, c0 + 2:c0 + 514],
                                     start=False, stop=True)
                ot = op.tile([128, W], DT, tag="ot")
                nc.scalar.copy(ot[:pout, 0:CA], ps[:pout, 0:CA])
                nc.vector.tensor_copy(ot[:pout, CA:W], ps[:pout, CA:W])
                nc.sync.dma_start(out=of[r0:r0 + pout, :], in_=ot[:pout])
    if legalize:
        _legalize_waits(nc)
    return nc


_CACHE = {}


def kernel(img: np.ndarray) -> np.ndarray:
    assert img.shape == (N_CORES * B, H, W), img.shape
    img16 = np.ascontiguousarray(np.asarray(img).astype(np.float16))
    if "nc" not in _CACHE:
        _CACHE["nc"] = _build()
        _CACHE["wpack"], _ = _weights_np()
    nc = _CACHE["nc"]
    pack = _CACHE["wpack"]
    in_maps = [{"x": img16[c * B:(c + 1) * B], "wpack": pack}
               for c in range(N_CORES)]
    res = run_bass_kernel_spmd(nc, in_maps, core_ids=list(range(N_CORES)))
    out = np.concatenate([res.results[c]["y"] for c in range(N_CORES)], axis=0)
    return out.astype(np.float32)


# revision 8
# speedup vs baseline: 16.2086x; 16.2086x over previous
"""Trainium2 Bass kernel for nn_Conv2Central (S^4 separable stencil), v2.

The reference computes y = S(rev(S(S(rev(S(x)))))) where S is the 2x2
stencil  out[i,j] = x[i,j] + .5 x[i,j+1] + .5 x[i+1,j] + .25 x[i+1,j+1]
(zero-padded bottom/right) applied per image, and rev reverses the batch.
S commutes with batch permutations, so the flips cancel: the net op is
S^4 — a separable 5-tap filter K5 = [1, 2, 1.5, 0.5, 0.0625] applied
along H then W (zero-padded bottom/right).

Sharding: batch N=32 split across 8 NeuronCores (4 images/core), no
inter-core communication.

v2 strategy (DMA-roofline bound, fp16 I/O):
  - I/O in fp16 (host casts; rounding ~2^-11 ≪ the 2e-2 gate), which
    halves HBM traffic vs fp32: ~16 MiB/core -> ~52 us at ~330 GB/s.
  - Horizontal K5 = [1,1,0.25] ⊛ [1,1,0.25] ([1,.5]^2 squared):
      t = x + x(+1)          DVE tensor_add (fp16 2x_1p mode)
      u = 0.25 * x(+2)       exact pow2 scale, split ACT (scalar.mul)
                             and Pool (gpsimd.tensor_scalar_mul)
      g = t + u              DVE tensor_add
  - Vertical K5 x horizontal [1,1,0.25] on PE: 3 banded fp16 matmuls
    per 512-col half accumulating in fp32 PSUM, stationary weights
    V_c and 0.25*V_c (V[k,m] = K5[k-m], zeroed across image
    boundaries).
  - PSUM -> SBUF fp16 copy split across ACT and Pool, then DMA out.
Tiles: 128 input rows -> 124 output rows (stride 124), final tile 128.
"""
import numpy as np

import concourse.bass as bass
import concourse.mybir as mybir
from concourse.tile import TileContext
from concourse.bass_utils import run_bass_kernel_spmd
from bass_rust import ScopedClock

N_CORES = 8
B = 4            # images per core
H = 1024
W = 1024
STRIDE = 124
K5 = [1.0, 2.0, 1.5, 0.5, 0.0625]
U_ACT_COLS = 400     # of W+2 u-columns, how many ACT computes (rest Pool)
CP_ACT_COLS = 924    # of W output columns ACT copies (rest DVE; Pool
                     # cannot read PSUM)

# ---------------------------------------------------------------------------
# Workarounds for this container's walrus build, which rejects any
# instruction carrying more than ONE sync wait ("Too many sync wait
# commands").  (1) TileContext's tail drain aggregates a wait per live
# semaphore — replace it with a chain of sync NOPs, one wait each.
# (2) A general pass splits any remaining multi-wait instruction by
# hoisting extra waits onto same-engine NoOps inserted right before it
# (engine queues are FIFO, so the waits still complete first).
# ---------------------------------------------------------------------------


def _patched_drain_and_barrier(self, tick_clock, wait_clock):
    nc = self.nc
    probe = nc.sync.nop()
    wait_clock.add_sem_waits(probe.ins, ScopedClock({None: tick_clock.global_clock}))
    si = probe.ins.sync_info
    waits = list(si.on_wait) if si and si.on_wait else []
    if si is not None:
        si.on_wait = waits[:1]
    for i in range(1, len(waits)):
        n = nc.sync.nop()
        nsi = n.ins.sync_info
        if nsi is None:
            n.ins.sync_info = mybir.SyncInfo(on_wait=[waits[i]], on_update=[])
        else:
            nsi.on_wait = [waits[i]]
    nc.sync.drain()
    nc.all_engine_barrier()
    assert self.sems is not None
    popped = nc._tile_sem_poison_stack.pop()
    assert popped is self._sem_poison
    nc.clear_and_free_semaphores(list(self.sems.allocated().values()))
    nc.all_engine_barrier()


TileContext._drain_and_barrier = _patched_drain_and_barrier

_nop_counter = [0]


def _legalize_waits(nc):
    for f in nc.m.functions:
        for blk in f.blocks:
            out = []
            for inst in blk.instructions:
                si = inst.sync_info
                waits = list(si.on_wait) if si is not None and si.on_wait else []
                if len(waits) > 1:
                    for w in waits[:-1]:
                        _nop_counter[0] += 1
                        nop = mybir.InstNoOp(name=f"legalize-wait-{_nop_counter[0]}")
                        nop.engine = inst.engine
                        nop.sync_info = mybir.SyncInfo(on_wait=[w], on_update=[])
                        out.append(nop)
                    si.on_wait = [waits[-1]]
                out.append(inst)
            blk.instructions = out
    return nc


# ---------------------------------------------------------------------------
# Weights: banded vertical-filter matrices.
# ---------------------------------------------------------------------------


def _band_np(rows_in, rows_out, boundary=None):
    """A[k, m] = K5[k-m], zeroed where out-row m and in-row k straddle
    `boundary` (tile-local image split)."""
    A = np.zeros((rows_in, rows_out), dtype=np.float32)
    for m in range(rows_out):
        for d in range(5):
            k = m + d
            if k < rows_in and not (boundary is not None and m < boundary <= k):
                A[k, m] = K5[d]
    return A


def _tile_plan():
    """[(r0, pin, pout, boundary_or_None)] covering B*H rows."""
    total = B * H
    plan = []
    r0 = 0
    while r0 < total:
        if total - r0 <= 128:
            plan.append((r0, total - r0, total - r0, None))
            break
        boundary = None
        for k in range(1, B):
            if r0 < k * H < r0 + 128:
                boundary = k * H - r0
        plan.append((r0, 128, STRIDE, boundary))
        r0 += STRIDE
    return plan


def _weights_np():
    plan = _tile_plan()
    classes = sorted({b for (_, _, _, b) in plan if b is not None})
    cols = []
    offs = {}

    def add(name, arr):
        offs[name] = sum(c.shape[1] for c in cols)
        cols.append(arr)

    add("main", _band_np(128, 128))
    add("mainq", 0.25 * _band_np(128, 128))
    for b in classes:
        add(f"main{b}", _band_np(128, STRIDE, boundary=b))
        add(f"mainq{b}", 0.25 * _band_np(128, STRIDE, boundary=b))
    return np.concatenate(cols, axis=1).astype(np.float16), offs


# ---------------------------------------------------------------------------
# Kernel builder.
# ---------------------------------------------------------------------------


def _build(reps=1, legalize=True):
    nc = bass.Bass(trn_type="TRN2")
    DT = mybir.dt.float16
    F32 = mybir.dt.float32
    pack, offs = _weights_np()
    x = nc.dram_tensor("x", [B, H, W], DT, kind="ExternalInput")
    wp = nc.dram_tensor("wpack", list(pack.shape), DT, kind="ExternalInput")
    y = nc.dram_tensor("y", [B, H, W], DT, kind="ExternalOutput")
    xf = x.rearrange("b h w -> (b h) w")
    yf = y.rearrange("b h w -> (b h) w")
    if reps > 1:
        scratch = nc.dram_tensor("scratch", [B, H, W], DT, kind="ExternalOutput")
        sf = scratch.rearrange("b h w -> (b h) w")

    plan = _tile_plan()
    UA, CA = U_ACT_COLS, CP_ACT_COLS

    with TileContext(nc) as tc:
        with tc.tile_pool(name="wpool", bufs=1) as wpool, \
             tc.tile_pool(name="xp", bufs=8) as xp, \
             tc.tile_pool(name="hp", bufs=9) as hp, \
             tc.tile_pool(name="op", bufs=8) as op, \
             tc.tile_pool(name="pp", bufs=4, space="PSUM") as pp:
            wt = wpool.tile(list(pack.shape), DT)
            nc.sync.dma_start(out=wt[:], in_=wp[:])

            def wslice(name, pin, pout):
                o = offs[name]
                return wt[:pin, o:o + pout]

            def drain(pend, of):
                # Late stage of a previous tile: PSUM -> SBUF (fp16) on
                # ACT + a DVE sliver, then DMA out. Emitted one iteration
                # late so in-order engine queues never stall the next
                # tile's early stage behind PE completion.
                r0, pout, ps = pend
                ot = op.tile([128, W], DT, tag="ot")
                nc.scalar.copy(ot[:pout, 0:CA], ps[:pout, 0:CA])
                nc.vector.tensor_copy(ot[:pout, CA:W], ps[:pout, CA:W])
                nc.sync.dma_start(out=of[r0:r0 + pout, :], in_=ot[:pout])

            pend = None
            pend_of = None
            for rep in range(reps):
              of = yf if rep == 0 else sf
              for ti, (r0, pin, pout, bnd) in enumerate(plan):
                xt = xp.tile([128, W + 4], DT, tag="xt")
                nc.sync.dma_start(out=xt[:pin, 0:W], in_=xf[r0:r0 + pin, :])
                nc.vector.memset(xt[:pin, W:W + 4], 0)
                t = hp.tile([128, W + 2], DT, tag="t")
                u = hp.tile([128, W + 2], DT, tag="u")
                g = hp.tile([128, W + 2], DT, tag="g")
                # u = 0.25 * x(+2), exact pow2 scale; split ACT / Pool
                nc.scalar.mul(u[:pin, 0:UA], xt[:pin, 2:2 + UA], 0.25)
                nc.gpsimd.tensor_scalar_mul(
                    u[:pin, UA:W + 2], xt[:pin, 2 + UA:W + 4], 0.25)
                # t = x + x(+1); g = t + u  (fp16 2x_1p adds on DVE)
                nc.vector.tensor_add(
                    t[:pin, 0:W + 2], xt[:pin, 0:W + 2], xt[:pin, 1:W + 3])
                nc.vector.tensor_add(g[:pin], t[:pin], u[:pin])
                mname = "main" if bnd is None else f"main{bnd}"
                qname = "mainq" if bnd is None else f"mainq{bnd}"
                ps = pp.tile([128, W], F32, tag="ps")
                for h in range(2):
                    c0 = h * 512
                    nc.tensor.matmul(ps[:pout, c0:c0 + 512],
                                     wslice(mname, pin, pout),
                                     g[:pin, c0:c0 + 512],
                                     start=True, stop=False)
                    nc.tensor.matmul(ps[:pout, c0:c0 + 512],
                                     wslice(mname, pin, pout),
                                     g[:pin, c0 + 1:c0 + 513],
                                     start=False, stop=False)
                    nc.tensor.matmul(ps[:pout, c0:c0 + 512],
                                     wslice(qname, pin, pout),
                                     g[:pin, c0 + 2:c0 + 514],
                                     start=False, stop=True)
                if pend is not None:
                    drain(pend, pend_of)
                pend = (r0, pout, ps)
                pend_of = of
            drain(pend, pend_of)
    if legalize:
        _legalize_waits(nc)
    return nc


_CACHE = {}


def kernel(img: np.ndarray) -> np.ndarray:
    assert img.shape == (N_CORES * B, H, W), img.shape
    img16 = np.ascontiguousarray(np.asarray(img).astype(np.float16))
    if "nc" not in _CACHE:
        _CACHE["nc"] = _build()
        _CACHE["wpack"], _ = _weights_np()
    nc = _CACHE["nc"]
    pack = _CACHE["wpack"]
    in_maps = [{"x": img16[c * B:(c + 1) * B], "wpack": pack}
               for c in range(N_CORES)]
    res = run_bass_kernel_spmd(nc, in_maps, core_ids=list(range(N_CORES)))
    out = np.concatenate([res.results[c]["y"] for c in range(N_CORES)], axis=0)
    return out.astype(np.float32)
